# revision 1
# baseline (speedup 1.0000x reference)
"""Trainium2 Bass kernel for a 4-layer compressed model:

    for l in range(4):  x = x @ (base[l] + bitdelta[l] * mask[l])

x: [16, 4096] f32, base/mask: [4, 4096, 4096] f32, bitdelta: [4] f32.

Sharding (8 cores, tensor parallel on weight columns):
  core c owns columns [c*512, (c+1)*512) of every layer's weight.

Key ideas:
  * Weights are never reconstructed on chip: by linearity,
        x @ (base + bd*mask) = x @ base + bd * (x @ mask),
    so base and mask stream straight from HBM into the PE array as
    float32r (TF32 PE mode, 1 cycle/row) moving operands, accumulating
    into two PSUM banks; one fused DVE op combines them per layer.
  * Contraction order is permuted to d = p*32 + k (p = SBUF partition,
    k = matmul index). The host lays weight shards out as [L, 8, 128,
    2048] so every 1 MiB weight DMA is one fully contiguous DRAM block,
    and the activation x^T [4096, 16] loads land partition-contiguous
    (one 2 KiB run per partition) with no rearrangement cost.
  * Between layers the [16,512] local result is PE-transposed to
    [512,16] and AllGather'd on the partition axis into the next
    layer's x^T — exactly the lhsT layout the next matmuls need.

Memory-bound: each core streams 64 MiB of weights; roofline ~180 us.
"""

import numpy as np

import concourse.bass as bass
import concourse.mybir as mybir
import concourse.tile as tile
from concourse import bacc
from concourse.bass_utils import run_bass_kernel_spmd
from concourse.masks import make_identity

L = 4
D = 4096
B = 16
NCORES = 8
C = D // NCORES          # 512 columns per core
KT = D // 128            # 32 contraction tiles of 128
GK = 4                   # k-tiles per weight DMA (1 MiB transfers)
NG = KT // GK            # 8 weight DMAs per tensor per layer
CT = C // 128            # 4 transpose chunks
WBUFS = 10               # weight tiles in flight per tensor (10 MiB)

F32 = mybir.dt.float32
F32R = mybir.dt.float32r
ALU = mybir.AluOpType

_cache = {}


def build():
    nc = bacc.Bacc(
        "TRN2",
        target_bir_lowering=False,
        debug=False,
        num_devices=NCORES,
    )

    # x^T in natural [4096, 16] order; row d = p*KT + k maps to SBUF
    # partition p, matmul index k — so the load is partition-contiguous.
    xT0 = nc.dram_tensor("xT0", [D, B], F32R, kind="ExternalInput")
    # weight shards, pre-permuted on host: [l, g, p, j*C+c] = W_l[p*KT+g*GK+j,
    # c]; each [128, GK*C] block is 1 MiB contiguous.
    base_sh = nc.dram_tensor("base_sh", [L, NG, 128, GK * C], F32R,
                             kind="ExternalInput")
    mask_sh = nc.dram_tensor("mask_sh", [L, NG, 128, GK * C], F32R,
                             kind="ExternalInput")
    bdb = nc.dram_tensor("bdb", [B, L], F32, kind="ExternalInput")
    out = nc.dram_tensor("out", [B, C], F32, kind="ExternalOutput")

    rg = [list(range(NCORES))]

    with tile.TileContext(nc) as tc:
        with (
            tc.tile_pool(name="w", bufs=WBUFS) as wpool,
            tc.tile_pool(name="xp", bufs=2) as xpool,
            tc.tile_pool(name="sp", bufs=2) as spool,
            tc.tile_pool(name="const", bufs=1) as cpool,
            tc.tile_pool(name="acc", bufs=2, space="PSUM") as psum,
            tc.tile_pool(name="tp", bufs=4, space="PSUM") as tpsum,
            tc.tile_pool(name="dram", bufs=2, space="DRAM") as dram,
        ):
            bd_sb = cpool.tile([B, L], F32, tag="bd")
            nc.scalar.dma_start(bd_sb[:, :], bdb[:, :])
            ident = cpool.tile([B, B], F32, tag="ident")
            make_identity(nc, ident[:, :])

            # Warmup AllGather: absorbs cross-core start skew and ncfw
            # warmup off the critical path (collectives run on TOPSP,
            # concurrent with the layer-0 weight stream).
            warm_in = dram.tile([1, L], F32, tag="warm_in")
            warm_out = dram.tile([NCORES, L], F32, tag="warm_out",
                                 addr_space="Shared")
            nc.gpsimd.dma_start(warm_in[:, :], bdb[0:1, :])
            nc.gpsimd.collective_compute(
                "AllGather",
                ALU.bypass,
                replica_groups=rg,
                ins=[warm_in.opt()],
                outs=[warm_out.opt()],
            )

            # xt[p, k*16+b] = x^T[p*KT + k, b]; one 2 KiB run per partition.
            xt = xpool.tile([128, KT * B], F32R, tag="xt")
            nc.scalar.dma_start(
                xt[:, :].rearrange("p (k b) -> p k b", k=KT),
                xT0[:, :].rearrange("(p k) b -> p k b", p=128),
            )

            for l in range(L):
                acc_b = psum.tile([B, C], F32, tag="accb")
                acc_m = psum.tile([B, C], F32, tag="accm")
                for g in range(NG):
                    wb = wpool.tile([128, GK * C], F32R, tag="wb")
                    nc.sync.dma_start(wb[:, :], base_sh[l, g])
                    wm = wpool.tile([128, GK * C], F32R, tag="wm")
                    nc.sync.dma_start(wm[:, :], mask_sh[l, g])
                    for j in range(GK):
                        k = g * GK + j
                        lhsT = xt[:, k * B:(k + 1) * B]
                        nc.tensor.matmul(
                            acc_b[:, :],
                            lhsT,
                            wb[:, j * C:(j + 1) * C],
                            start=(k == 0),
                            stop=(k == KT - 1),
                        )
                        nc.tensor.matmul(
                            acc_m[:, :],
                            lhsT,
                            wm[:, j * C:(j + 1) * C],
                            start=(k == 0),
                            stop=(k == KT - 1),
                        )

                # y = acc_b + bitdelta[l] * acc_m  (DVE can read only one
                # PSUM operand, so stage acc_b through SBUF on ScalarE)
                yb_sb = spool.tile([B, C], F32, tag="yb")
                nc.scalar.copy(yb_sb[:, :], acc_b[:, :])
                y_sb = spool.tile([B, C], F32, tag="y")
                nc.vector.scalar_tensor_tensor(
                    out=y_sb[:, :],
                    in0=acc_m[:, :],
                    scalar=bd_sb[:, l:l + 1],
                    in1=yb_sb[:, :],
                    op0=ALU.mult,
                    op1=ALU.add,
                )

                if l == L - 1:
                    nc.scalar.dma_start(out[:, :], y_sb[:, :])
                else:
                    # y [16, 512] -> y^T [512, 16] via 4 PE transposes,
                    # then AllGather into the next layer's x^T [4096, 16].
                    yt_sb = spool.tile([128, CT * B], F32, tag="yt")
                    for cc in range(CT):
                        pt = tpsum.tile([128, B], F32, tag="pt")
                        nc.tensor.transpose(
                            pt[:, :],
                            y_sb[:, cc * 128:(cc + 1) * 128],
                            ident[:, :],
                        )
                        nc.vector.tensor_copy(
                            yt_sb[:, cc * B:(cc + 1) * B], pt[:, :]
                        )
                    ytb = dram.tile([C, B], F32R, tag="ytb")
                    nc.gpsimd.dma_start(
                        ytb[:, :].rearrange("(cc p) b -> p cc b", p=128),
                        yt_sb[:, :].rearrange("p (cc b) -> p cc b", cc=CT),
                    )
                    xt_full = dram.tile([D, B], F32R, tag="xtf",
                                        addr_space="Shared")
                    nc.gpsimd.collective_compute(
                        "AllGather",
                        ALU.bypass,
                        replica_groups=rg,
                        ins=[ytb.opt()],
                        outs=[xt_full.opt()],
                    )
                    xt = xpool.tile([128, KT * B], F32R, tag="xt")
                    nc.scalar.dma_start(
                        xt[:, :].rearrange("p (k b) -> p k b", k=KT),
                        xt_full[:, :].rearrange("(p k) b -> p k b", p=128),
                    )

    nc.compile()
    return nc


def _get_nc():
    if "nc" not in _cache:
        _cache["nc"] = build()
    return _cache["nc"]


def _shard_weight(w):
    """[L, D, C] column shard -> [L, NG, 128, GK*C] with
    out[l, g, p, j*C + c] = w[l, p*KT + g*GK + j, c]."""
    w = w.reshape(L, 128, NG, GK, C)
    w = w.transpose(0, 2, 1, 3, 4)            # [L, NG, 128, GK, C]
    return np.ascontiguousarray(w.reshape(L, NG, 128, GK * C))


def _make_in_maps(x, base, mask, bitdelta):
    x = np.ascontiguousarray(x, dtype=np.float32)
    base = np.ascontiguousarray(base, dtype=np.float32)
    mask = np.ascontiguousarray(mask, dtype=np.float32)
    bitdelta = np.ascontiguousarray(bitdelta, dtype=np.float32)

    xT = np.ascontiguousarray(x.T)                       # [D, B]
    bdb = np.broadcast_to(bitdelta[None, :], (B, L)).copy()

    in_maps = []
    for c in range(NCORES):
        sl = slice(c * C, (c + 1) * C)
        in_maps.append({
            "xT0": xT,
            "base_sh": _shard_weight(base[:, :, sl]),
            "mask_sh": _shard_weight(mask[:, :, sl]),
            "bdb": bdb,
        })
    return in_maps


def _run(x, base, mask, bitdelta, trace=False):
    nc = _get_nc()
    in_maps = _make_in_maps(x, base, mask, bitdelta)
    res = run_bass_kernel_spmd(
        nc, in_maps, core_ids=list(range(NCORES)), trace=trace
    )
    y = np.concatenate([res.results[c]["out"] for c in range(NCORES)], axis=1)
    return y, res


def kernel(x, base, mask, bitdelta):
    y, _ = _run(x, base, mask, bitdelta)
    return y



# revision 5
# speedup vs baseline: 2.0125x; 2.0125x over previous
"""Trainium2 Bass kernel for a 4-layer compressed model:

    for l in range(4):  x = x @ (base[l] + bitdelta[l] * mask[l])

x: [16, 4096] f32, base/mask: [4, 4096, 4096] f32, bitdelta: [4] f32.

Strategy (8 cores):
  * Weights stream as fp8e4: mask is a +/-1 sign tensor (EXACT in fp8);
    base is pre-scaled x64 on the host before fp8 quantization (keeps
    values out of the subnormal floor).  The x64 is folded into the
    per-layer combine scalar (64*bd) so every layer output is scaled by
    64; the host divides the final gather by 64^4.  This quarters HBM
    traffic vs f32 - the problem is memory-bound.
  * By linearity, x @ (base + bd*mask) = x @ base + bd * (x @ mask):
    base and mask stream straight into the PE as fp8 moving operands
    against a stationary x^T tile (f32r), accumulating into separate
    PSUM banks; one DVE scalar_tensor_tensor op combines them.
  * The network is linear end to end, so layers are paired
    column-parallel -> row-parallel: core c owns columns [c*512,(c+1)*512)
    of layer 2e and rows [c*512,(c+1)*512) of layer 2e+1.  The col->row
    hand-off is core-LOCAL (a 4x PE transpose, no DRAM round trip) and
    the row layer emits a [16, 4096] partial-sum shard.  There are NO
    on-device collectives (ncfw warmup costs ~55us/exec and AllGathers
    ~16us each - they would dominate at fp8 speed).  kernel() runs the
    same compiled program twice (layers 0/1 then 2/3); unsharding the
    row-parallel output (summing the 8 partials) happens on the host.
"""

import numpy as np

import concourse.bass as bass
import concourse.mybir as mybir
import concourse.tile as tile
from concourse import bacc
from concourse.bass_utils import run_bass_kernel_spmd
from concourse.masks import make_identity

D = 4096
B = 16
NCORES = 8
C = D // NCORES          # 512 cols (rows) per core per layer
KT = D // 128            # 32 contraction tiles in the col layer
GK = 8                   # col-layer k-tiles per weight DMA (512 KiB)
NG = KT // GK            # 4 weight DMAs per tensor for the col layer
RKT = C // 128           # 4 contraction tiles in the row layer
NC = D // C              # 8 output chunks of 512 in the row layer
SCALE = 64.0             # host pre-scale on base before fp8 quantization

F32 = mybir.dt.float32
F32R = mybir.dt.float32r
F8 = mybir.dt.float8e4
BF16 = mybir.dt.bfloat16
ALU = mybir.AluOpType

_cache = {}


def build():
    nc = bacc.Bacc(
        "TRN2",
        target_bir_lowering=False,
        debug=False,
        num_devices=NCORES,
    )

    # x^T [4096, 16]; row d = p*KT + k maps to SBUF partition p, matmul
    # index k, so the load is partition-contiguous (2 KiB runs).
    xT0 = nc.dram_tensor("xT0", [D, B], BF16, kind="ExternalInput")
    # col-layer weight shards, host-permuted: [g, p, j*C+c] =
    # W[p*KT + g*GK + j, c0+c]; each [128, GK*C] block is one 512 KiB
    # contiguous DMA.
    colb = nc.dram_tensor("colb", [NG, 128, GK * C], F8, kind="ExternalInput")
    colm = nc.dram_tensor("colm", [NG, 128, GK * C], F8, kind="ExternalInput")
    # row-layer weight shards: [kc, p, n] = W[c0 + kc*128 + p, n];
    # contiguous rows (they arrive from the local transpose in this
    # order), one 512 KiB DMA per kc.
    rowb = nc.dram_tensor("rowb", [RKT, 128, D], F8, kind="ExternalInput")
    rowm = nc.dram_tensor("rowm", [RKT, 128, D], F8, kind="ExternalInput")
    # combine scalars, broadcast over B rows: [:, 0] = 64*bd[l0],
    # [:, 1] = 64*bd[l1].
    bdb = nc.dram_tensor("bdb", [B, 2], F32, kind="ExternalInput")
    # row-parallel partial-sum output (full width).
    out = nc.dram_tensor("out", [B, D], F32, kind="ExternalOutput")

    with tile.TileContext(nc) as tc:
        with (
            tc.tile_pool(name="wc", bufs=2 * NG) as wcpool,
            tc.tile_pool(name="wr", bufs=2 * RKT) as wrpool,
            tc.tile_pool(name="xp", bufs=1) as xpool,
            tc.tile_pool(name="sp", bufs=4) as spool,
            tc.tile_pool(name="const", bufs=1) as cpool,
            tc.tile_pool(name="ps", bufs=1, space="PSUM") as psum,
        ):
            bd_sb = cpool.tile([B, 2], F32, tag="bd")
            nc.scalar.dma_start(bd_sb[:, :], bdb[:, :])
            ident = cpool.tile([B, B], F32, tag="ident")
            make_identity(nc, ident[:, :])

            # xt[p, k*16+b] = x^T[p*KT + k, b]
            xt = xpool.tile([128, KT * B], BF16, tag="xt")
            nc.scalar.dma_start(
                xt[:, :].rearrange("p (k b) -> p k b", k=KT),
                xT0[:, :].rearrange("(p k) b -> p k b", p=128),
            )

            # ---- column-parallel layer: y0[16, 512] = x @ Wcol ----
            wbt, wmt = [], []
            for g in range(NG):
                wb = wcpool.tile([128, GK * C], F8, tag="wcb")
                nc.sync.dma_start(wb[:, :], colb[g])
                wm = wcpool.tile([128, GK * C], F8, tag="wcm")
                nc.sync.dma_start(wm[:, :], colm[g])
                wbt.append(wb)
                wmt.append(wm)
            # prefetch row-layer weights right behind (deep SBUF buffer,
            # DMA never idles across the layer boundary)
            rbt, rmt = [], []
            for kc in range(RKT):
                rb = wrpool.tile([128, D], F8, tag="wrb")
                nc.sync.dma_start(rb[:, :], rowb[kc])
                rm = wrpool.tile([128, D], F8, tag="wrm")
                nc.sync.dma_start(rm[:, :], rowm[kc])
                rbt.append(rb)
                rmt.append(rm)

            acc = psum.tile([B, 2 * C], F32, tag="acc")  # b: bank0, m: bank1
            for g in range(NG):
                for j in range(GK):
                    k = g * GK + j
                    lhsT = xt[:, k * B:(k + 1) * B]
                    nc.tensor.matmul(
                        acc[:, 0:C], lhsT, wbt[g][:, j * C:(j + 1) * C],
                        start=(k == 0), stop=(k == KT - 1),
                    )
                    nc.tensor.matmul(
                        acc[:, C:2 * C], lhsT, wmt[g][:, j * C:(j + 1) * C],
                        start=(k == 0), stop=(k == KT - 1),
                    )

            # y0 = acc_b + (64*bd0) * acc_m   (DVE reads one PSUM operand;
            # stage acc_b through SBUF on ScalarE)
            yb = spool.tile([B, C], F32, tag="yb")
            nc.scalar.copy(yb[:, :], acc[:, 0:C])
            y0 = spool.tile([B, C], F32, tag="y0")
            nc.vector.scalar_tensor_tensor(
                out=y0[:, :], in0=acc[:, C:2 * C], scalar=bd_sb[:, 0:1],
                in1=yb[:, :], op0=ALU.mult, op1=ALU.add,
            )

            # ---- local transpose: y0 [16,512] -> xt1 [128, 4*16] ----
            tp = psum.tile([128, RKT * B], F32, tag="tp")
            for cc in range(RKT):
                nc.tensor.transpose(
                    tp[:, cc * B:(cc + 1) * B],
                    y0[:, cc * 128:(cc + 1) * 128],
                    ident[:, :],
                )
            xt1 = spool.tile([128, RKT * B], BF16, tag="xt1")
            nc.vector.tensor_copy(xt1[:, :], tp[:, :])

            # ---- row-parallel layer: partial[16, 4096] = y0 @ Wrow ----
            # four 1024-wide quarters (PSUM: 2 banks per tensor per
            # quarter; whole kernel stays within the 8 banks)
            QW = D // 4
            for h in range(4):
                ab = psum.tile([B, QW], F32, tag="rab")
                am = psum.tile([B, QW], F32, tag="ram")
                for kc in range(RKT):
                    lhsT = xt1[:, kc * B:(kc + 1) * B]
                    for q in range(QW // C):
                        n0 = h * QW + q * C
                        nc.tensor.matmul(
                            ab[:, q * C:(q + 1) * C], lhsT,
                            rbt[kc][:, n0:n0 + C],
                            start=(kc == 0), stop=(kc == RKT - 1),
                        )
                        nc.tensor.matmul(
                            am[:, q * C:(q + 1) * C], lhsT,
                            rmt[kc][:, n0:n0 + C],
                            start=(kc == 0), stop=(kc == RKT - 1),
                        )
                for q in range(QW // C):
                    n0 = h * QW + q * C
                    yb2 = spool.tile([B, C], F32, tag="yb2")
                    nc.scalar.copy(yb2[:, :], ab[:, q * C:(q + 1) * C])
                    y1 = spool.tile([B, C], F32, tag="y1")
                    nc.vector.scalar_tensor_tensor(
                        out=y1[:, :], in0=am[:, q * C:(q + 1) * C],
                        scalar=bd_sb[:, 1:2], in1=yb2[:, :],
                        op0=ALU.mult, op1=ALU.add,
                    )
                    nc.scalar.dma_start(out[:, n0:n0 + C], y1[:, :])

    nc.compile()
    return nc


def _get_nc():
    if "nc" not in _cache:
        _cache["nc"] = build()
    return _cache["nc"]


def _fp8(a):
    import ml_dtypes

    return np.clip(a, -240.0, 240.0).astype(ml_dtypes.float8_e4m3)


def _shard_col(w):
    """[D, C] col shard -> [NG, 128, GK*C] with
    out[g, p, j*C + c] = w[p*KT + g*GK + j, c]."""
    w = w.reshape(128, NG, GK, C)
    return np.ascontiguousarray(w.transpose(1, 0, 2, 3).reshape(NG, 128, GK * C))


def _make_in_maps(xT, w0, w1, bd0, bd1):
    """One exec = col layer (w0) + row layer (w1). xT: [D, B]."""
    import ml_dtypes
    bdb = np.broadcast_to(
        np.array([bd0 * SCALE, bd1 * SCALE], np.float32)[None, :], (B, 2)
    ).copy()
    in_maps = []
    for c in range(NCORES):
        sl = slice(c * C, (c + 1) * C)
        in_maps.append({
            "xT0": np.ascontiguousarray(xT).astype(ml_dtypes.bfloat16),
            "colb": _shard_col(_fp8(w0[0][:, sl] * SCALE)),
            "colm": _shard_col(_fp8(w0[1][:, sl])),
            "rowb": _fp8(w1[0][sl, :] * SCALE).reshape(RKT, 128, D),
            "rowm": _fp8(w1[1][sl, :]).reshape(RKT, 128, D),
            "bdb": bdb,
        })
    return in_maps


def _run(x, base, mask, bitdelta, trace=False):
    nc = _get_nc()
    base = np.asarray(base, np.float32)
    mask = np.asarray(mask, np.float32)
    bd = np.asarray(bitdelta, np.float32)

    results = []
    xT = np.ascontiguousarray(np.asarray(x, np.float32).T)
    for e in range(2):
        l0, l1 = 2 * e, 2 * e + 1
        in_maps = _make_in_maps(
            xT, (base[l0], mask[l0]), (base[l1], mask[l1]), bd[l0], bd[l1]
        )
        res = run_bass_kernel_spmd(
            nc, in_maps, core_ids=list(range(NCORES)), trace=trace
        )
        # unshard the row-parallel output: sum the 8 partial shards
        y = np.sum([res.results[c]["out"] for c in range(NCORES)], axis=0)
        results.append(res)
        xT = np.ascontiguousarray(y.T)
    return y / SCALE ** 4, results


def kernel(x, base, mask, bitdelta):
    y, _ = _run(x, base, mask, bitdelta)
    return y.astype(np.float32)


# revision 12
# speedup vs baseline: 2.0626x; 1.0249x over previous
"""Trainium2 Bass kernel for a 4-layer compressed model:

    for l in range(4):  x = x @ (base[l] + bitdelta[l] * mask[l])

x: [16, 4096] f32, base/mask: [4, 4096, 4096] f32, bitdelta: [4] f32.

Strategy (8 cores):
  * Weights stream as fp8e4: mask is a +/-1 sign tensor (EXACT in fp8);
    base is pre-scaled x64 on the host before fp8 quantization (keeps
    values out of the subnormal floor).  The x64 is folded into the
    per-layer combine scalar (64*bd) so every layer output is scaled by
    64; the host divides the final gather by 64^4.  This quarters HBM
    traffic vs f32 - the problem is memory-bound.
  * By linearity, x @ (base + bd*mask) = x @ base + bd * (x @ mask):
    base and mask stream straight into the PE as fp8 moving operands
    against a stationary x^T tile (bf16), accumulating into separate
    PSUM banks; one DVE scalar_tensor_tensor op combines them.
  * The network is linear end to end, so layers are paired
    column-parallel -> row-parallel: core c owns columns [c*512,(c+1)*512)
    of layer 2e and rows [c*512,(c+1)*512) of layer 2e+1.  The col->row
    hand-off is core-LOCAL (a 4x PE transpose, no DRAM round trip) and
    the row layer emits a [16, 4096] partial-sum shard.  There are NO
    on-device collectives (ncfw warmup costs ~55us/exec and AllGathers
    ~16us each - they would dominate at fp8 speed).  kernel() runs the
    same compiled program twice (layers 0/1 then 2/3); unsharding the
    row-parallel output (summing the 8 partials) happens on the host.
  * Perf details: weight DMAs are spread over four hardware queues
    (sync/vector/gpsimd/scalar) so the descriptor ramp is 4x faster; a
    block of dummy N=16 matmuls during the initial DMA wait keeps the
    PE HAM clock-gate at K=8/8 (2.4 GHz) before the real stream; every
    matmul is issued as a 2-strip pair in PE column groups 0/64
    (tile_position), so two moving streams run concurrently and the PE
    stays ahead of DMA; row-layer PSUM quarters ping-pong to avoid
    write-after-read stalls.
"""

import numpy as np

import concourse.bass as bass
import concourse.mybir as mybir
import concourse.tile as tile
from concourse import bacc
from concourse.bass_utils import run_bass_kernel_spmd
from concourse.masks import make_identity

D = 4096
B = 16
NCORES = 8
C = D // NCORES          # 512 cols (rows) per core per layer
KT = D // 128            # 32 contraction tiles in the col layer
GK = 4                   # col-layer k-tiles per weight DMA (256 KiB)
NG = KT // GK            # 8 weight DMAs per tensor for the col layer
RKT = C // 128           # 4 contraction tiles in the row layer
SCALE = 64.0             # host pre-scale on base before fp8 quantization
NWARM = 72               # PE warm-up matmuls (~3.5us to open the HAM gate)

F32 = mybir.dt.float32
F8 = mybir.dt.float8e4
BF16 = mybir.dt.bfloat16
ALU = mybir.AluOpType

_cache = {}


def build():
    nc = bacc.Bacc(
        "TRN2",
        target_bir_lowering=False,
        debug=False,
        num_devices=NCORES,
    )

    # x^T [4096, 16]; row d = p*KT + k maps to SBUF partition p, matmul
    # index k, so the load is partition-contiguous.
    xT0 = nc.dram_tensor("xT0", [D, B], BF16, kind="ExternalInput")
    # col-layer weight shards, host-permuted: [g, p, j*C+c] =
    # W[p*KT + g*GK + j, c0+c]; each [128, GK*C] block is one contiguous
    # 256 KiB DMA.
    colb = nc.dram_tensor("colb", [NG, 128, GK * C], F8, kind="ExternalInput")
    colm = nc.dram_tensor("colm", [NG, 128, GK * C], F8, kind="ExternalInput")
    # row-layer weight shards: [kc, p, n] = W[c0 + kc*128 + p, n];
    # contiguous rows (they arrive from the local transpose in this
    # order), one 512 KiB DMA per kc.
    rowb = nc.dram_tensor("rowb", [RKT, 128, D], F8, kind="ExternalInput")
    rowm = nc.dram_tensor("rowm", [RKT, 128, D], F8, kind="ExternalInput")
    # combine scalars, broadcast over B rows: [:, 0] = 64*bd[l0],
    # [:, 1] = 64*bd[l1].
    bdb = nc.dram_tensor("bdb", [B, 2], F32, kind="ExternalInput")
    # row-parallel partial-sum output (full width).
    out = nc.dram_tensor("out", [B, D], F32, kind="ExternalOutput")

    with tile.TileContext(nc) as tc:
        with (
            tc.tile_pool(name="wc", bufs=2 * NG) as wcpool,
            tc.tile_pool(name="wr", bufs=2 * RKT) as wrpool,
            tc.tile_pool(name="xp", bufs=1) as xpool,
            tc.tile_pool(name="sp", bufs=4) as spool,
            tc.tile_pool(name="const", bufs=1) as cpool,
            tc.tile_pool(name="ps", bufs=1, space="PSUM") as psum,
        ):
            bd_sb = cpool.tile([128, 2], F32, tag="bd")
            nc.scalar.dma_start(bd_sb[0:B, :], bdb[:, :])
            nc.scalar.dma_start(bd_sb[64:64 + B, :], bdb[:, :])
            ident = cpool.tile([128, B], F32, tag="ident")
            make_identity(nc, ident[0:B, :])
            make_identity(nc, ident[64:64 + B, :])

            # ---- PE warm-up: dummy matmuls while the first weight DMAs
            # are in flight, so HAM reaches K=8/8 before the real stream.
            warm = cpool.tile([128, B], BF16, tag="warm")
            nc.vector.memset(warm[:, :], 0.0)
            wps = psum.tile([B, B], F32, tag="wps")
            for _ in range(NWARM):
                nc.tensor.matmul(wps[:, :], warm[:, :], warm[:, :],
                                 start=True, stop=True)

            # xt[p, k*16+b] = x^T[p*KT + k, b]
            xt = xpool.tile([128, KT * B], BF16, tag="xt")
            nc.scalar.dma_start(
                xt[:, :].rearrange("p (k b) -> p k b", k=KT),
                xT0[:, :].rearrange("(p k) b -> p k b", p=128),
            )

            # weight DMAs: one hardware queue per tensor stream
            wbt, wmt = [], []
            for g in range(NG):
                wb = wcpool.tile([128, GK * C], F8, tag="wcb")
                nc.sync.dma_start(wb[:, :], colb[g])
                wm = wcpool.tile([128, GK * C], F8, tag="wcm")
                nc.gpsimd.dma_start(wm[:, :], colm[g])
                wbt.append(wb)
                wmt.append(wm)
            rbt, rmt = [], []
            for kc in range(RKT):
                rb = wrpool.tile([128, D], F8, tag="wrb")
                nc.gpsimd.dma_start(rb[:, :], rowb[kc])
                rm = wrpool.tile([128, D], F8, tag="wrm")
                nc.scalar.dma_start(rm[:, :], rowm[kc])
                rbt.append(rb)
                rmt.append(rm)

            # ---- column-parallel layer: y0[16, 512] = x @ Wcol ----
            # 2-strip PE col tiling: strip s covers cols [s*256, s*256+256)
            # in PE col group 64*s, PSUM partitions [64*s, 64*s+16).
            # acc bank layout: b at cols [0,256), m at cols [256,512).
            cacc = psum.tile([128, 1024], F32, tag="cacc")  # b: bank0, m: bank1
            HC = 256
            for g in range(NG):
                for j in range(GK):
                    k = g * GK + j
                    lhsT = xt[:, k * B:(k + 1) * B]
                    for s in range(2):
                        p0 = 64 * s
                        nc.tensor.matmul(
                            cacc[p0:p0 + B, 0:HC], lhsT,
                            wbt[g][:, j * C + s * HC:j * C + (s + 1) * HC],
                            start=(k == 0), stop=(k == KT - 1),
                            tile_position=(0, p0), skip_group_check=True,
                        )
                        nc.tensor.matmul(
                            cacc[p0:p0 + B, 512:512 + HC], lhsT,
                            wmt[g][:, j * C + s * HC:j * C + (s + 1) * HC],
                            start=(k == 0), stop=(k == KT - 1),
                            tile_position=(0, p0), skip_group_check=True,
                        )

            # y0 = acc_b + (64*bd0) * acc_m, per strip (partitions 0/64)
            y0 = spool.tile([128, HC], F32, tag="y0")
            for s in range(2):
                p0 = 64 * s
                yb = spool.tile([128, HC], F32, tag="yb")
                nc.scalar.copy(yb[p0:p0 + B, :], cacc[p0:p0 + B, 0:HC])
                nc.vector.scalar_tensor_tensor(
                    out=y0[p0:p0 + B, :], in0=cacc[p0:p0 + B, 512:512 + HC],
                    scalar=bd_sb[p0:p0 + B, 0:1], in1=yb[p0:p0 + B, :],
                    op0=ALU.mult, op1=ALU.add,
                )

            # ---- local transpose: y0 -> xt1 [128, 4*16] ----
            # strip s holds y0 cols [s*256,(s+1)*256) on partitions 64s..
            tp = psum.tile([128, RKT * B], F32, tag="tp")
            for cc in range(RKT):
                s, half = divmod(cc, 2)
                p0 = 64 * s
                nc.tensor.transpose(
                    tp[:, cc * B:(cc + 1) * B],
                    y0[p0:p0 + B, half * 128:(half + 1) * 128],
                    ident[p0:p0 + B, :],
                )
            xt1 = spool.tile([128, RKT * B], BF16, tag="xt1")
            nc.vector.tensor_copy(xt1[:, :], tp[:, :])

            # ---- row-parallel layer: partial[16, 4096] = y0 @ Wrow ----
            # quarter h covers cols [h*1024, h*1024+1024): strip s does the
            # 512-chunk h*2+s in PE col group 64*s.  PSUM quarters
            # ping-pong (tags rq0/rq1) so quarter h+1 never waits on the
            # combine of quarter h.
            qacc = [
                psum.tile([128, C], F32, tag=f"rq{i}", name=f"rq{i}")
                for i in range(2)
            ]
            macc = [
                psum.tile([128, C], F32, tag=f"mq{i}", name=f"mq{i}")
                for i in range(2)
            ]
            for h in range(4):
                ab, am = qacc[h % 2], macc[h % 2]
                for kc in range(RKT):
                    lhsT = xt1[:, kc * B:(kc + 1) * B]
                    for s in range(2):
                        p0 = 64 * s
                        n0 = (h * 2 + s) * C
                        nc.tensor.matmul(
                            ab[p0:p0 + B, :], lhsT, rbt[kc][:, n0:n0 + C],
                            start=(kc == 0), stop=(kc == RKT - 1),
                            tile_position=(0, p0), skip_group_check=True,
                        )
                        nc.tensor.matmul(
                            am[p0:p0 + B, :], lhsT, rmt[kc][:, n0:n0 + C],
                            start=(kc == 0), stop=(kc == RKT - 1),
                            tile_position=(0, p0), skip_group_check=True,
                        )
                for s in range(2):
                    p0 = 64 * s
                    n0 = (h * 2 + s) * C
                    yb2 = spool.tile([128, C], F32, tag="yb2")
                    nc.scalar.copy(yb2[p0:p0 + B, :], ab[p0:p0 + B, :])
                    y1 = spool.tile([128, C], F32, tag="y1")
                    nc.vector.scalar_tensor_tensor(
                        out=y1[p0:p0 + B, :], in0=am[p0:p0 + B, :],
                        scalar=bd_sb[p0:p0 + B, 1:2], in1=yb2[p0:p0 + B, :],
                        op0=ALU.mult, op1=ALU.add,
                    )
                    nc.sync.dma_start(out[:, n0:n0 + C], y1[p0:p0 + B, :])

    nc.compile()
    return nc


def _get_nc():
    if "nc" not in _cache:
        _cache["nc"] = build()
    return _cache["nc"]


def _fp8(a):
    import ml_dtypes

    return np.clip(a, -240.0, 240.0).astype(ml_dtypes.float8_e4m3)


def _shard_col(w):
    """[D, C] col shard -> [NG, 128, GK*C] with
    out[g, p, j*C + c] = w[p*KT + g*GK + j, c]."""
    w = w.reshape(128, NG, GK, C)
    return np.ascontiguousarray(w.transpose(1, 0, 2, 3).reshape(NG, 128, GK * C))


def _make_in_maps(xT, w0, w1, bd0, bd1):
    """One exec = col layer (w0) + row layer (w1). xT: [D, B]."""
    import ml_dtypes
    bdb = np.broadcast_to(
        np.array([bd0 * SCALE, bd1 * SCALE], np.float32)[None, :], (B, 2)
    ).copy()
    in_maps = []
    for c in range(NCORES):
        sl = slice(c * C, (c + 1) * C)
        in_maps.append({
            "xT0": np.ascontiguousarray(xT).astype(ml_dtypes.bfloat16),
            "colb": _shard_col(_fp8(w0[0][:, sl] * SCALE)),
            "colm": _shard_col(_fp8(w0[1][:, sl])),
            "rowb": _fp8(w1[0][sl, :] * SCALE).reshape(RKT, 128, D),
            "rowm": _fp8(w1[1][sl, :]).reshape(RKT, 128, D),
            "bdb": bdb,
        })
    return in_maps


def _run(x, base, mask, bitdelta, trace=False):
    nc = _get_nc()
    base = np.asarray(base, np.float32)
    mask = np.asarray(mask, np.float32)
    bd = np.asarray(bitdelta, np.float32)

    results = []
    xT = np.ascontiguousarray(np.asarray(x, np.float32).T)
    for e in range(2):
        l0, l1 = 2 * e, 2 * e + 1
        in_maps = _make_in_maps(
            xT, (base[l0], mask[l0]), (base[l1], mask[l1]), bd[l0], bd[l1]
        )
        res = run_bass_kernel_spmd(
            nc, in_maps, core_ids=list(range(NCORES)), trace=trace
        )
        # unshard the row-parallel output: sum the 8 partial shards
        y = np.sum([res.results[c]["out"] for c in range(NCORES)], axis=0)
        results.append(res)
        xT = np.ascontiguousarray(y.T)
    return y / SCALE ** 4, results


def kernel(x, base, mask, bitdelta):
    y, _ = _run(x, base, mask, bitdelta)
    return y.astype(np.float32)


# revision 13
# speedup vs baseline: 2.2026x; 1.0679x over previous
"""Trainium2 Bass kernel for a 4-layer compressed model:

    for l in range(4):  x = x @ (base[l] + bitdelta[l] * mask[l])

x: [16, 4096] f32, base/mask: [4, 4096, 4096] f32, bitdelta: [4] f32.

Strategy (8 cores):
  * Weights stream as fp8e4: mask is a +/-1 sign tensor (EXACT in fp8);
    base is pre-scaled x64 on the host before fp8 quantization (keeps
    values out of the subnormal floor).  The x64 is folded into the
    per-layer combine scalar (64*bd) so every layer output is scaled by
    64; the host divides the final gather by 64^4.  This quarters HBM
    traffic vs f32 - the problem is memory-bound.
  * By linearity, x @ (base + bd*mask) = x @ base + bd * (x @ mask):
    base and mask stream straight into the PE as fp8 moving operands
    against a stationary x^T tile (bf16), accumulating into separate
    PSUM banks; one DVE scalar_tensor_tensor op combines them.
  * The network is linear end to end, so layers are paired
    column-parallel -> row-parallel: core c owns columns [c*512,(c+1)*512)
    of layer 2e and rows [c*512,(c+1)*512) of layer 2e+1.  The col->row
    hand-off is core-LOCAL (a 4x PE transpose, no DRAM round trip) and
    the row layer emits a [16, 4096] partial-sum shard.  There are NO
    on-device collectives (ncfw warmup costs ~55us/exec and AllGathers
    ~16us each - they would dominate at fp8 speed).  kernel() runs the
    same compiled program twice (layers 0/1 then 2/3); unsharding the
    row-parallel output (summing the 8 partials) happens on the host.
  * Perf details: weight DMAs are spread over four hardware queues
    (sync/vector/gpsimd/scalar) so the descriptor ramp is 4x faster; a
    block of dummy N=16 matmuls during the initial DMA wait keeps the
    PE HAM clock-gate at K=8/8 (2.4 GHz) before the real stream; every
    matmul is issued as a 2-strip pair in PE column groups 0/64
    (tile_position), so two moving streams run concurrently and the PE
    stays ahead of DMA; row-layer PSUM quarters ping-pong to avoid
    write-after-read stalls.
"""

import numpy as np

import concourse.bass as bass
import concourse.mybir as mybir
import concourse.tile as tile
from concourse import bacc
from concourse.bass_utils import run_bass_kernel_spmd
from concourse.masks import make_identity

D = 4096
B = 16
NCORES = 8
C = D // NCORES          # 512 cols (rows) per core per layer
KT = D // 128            # 32 contraction tiles in the col layer
GK = 4                   # col-layer k-tiles per weight DMA (256 KiB)
NG = KT // GK            # 8 weight DMAs per tensor for the col layer
RKT = C // 128           # 4 contraction tiles in the row layer
SCALE = 64.0             # host pre-scale on base before fp8 quantization
NWARM = 55               # PE warm-up matmuls (~4us to open the HAM gate)

F32 = mybir.dt.float32
F8 = mybir.dt.float8e4
BF16 = mybir.dt.bfloat16
ALU = mybir.AluOpType

_cache = {}


def build():
    nc = bacc.Bacc(
        "TRN2",
        target_bir_lowering=False,
        debug=False,
        num_devices=NCORES,
    )

    # x^T [4096, 16]; row d = p*KT + k maps to SBUF partition p, matmul
    # index k, so the load is partition-contiguous.
    xT0 = nc.dram_tensor("xT0", [D, B], BF16, kind="ExternalInput")
    # col-layer weight shards, host-permuted: [g, p, j*C+c] =
    # W[p*KT + g*GK + j, c0+c]; each [128, GK*C] block is one contiguous
    # 256 KiB DMA.
    colb = nc.dram_tensor("colb", [NG, 128, GK * C], F8, kind="ExternalInput")
    colm = nc.dram_tensor("colm", [NG, 128, GK * C], F8, kind="ExternalInput")
    # row-layer weight shards, quarter-major so the first output quarter
    # can start before the rest arrive: [h, kc, p, m] =
    # W[c0 + kc*128 + p, h*1024 + m]; one 128 KiB DMA per (h, kc).
    rowb = nc.dram_tensor("rowb", [4, RKT, 128, D // 4], F8,
                          kind="ExternalInput")
    rowm = nc.dram_tensor("rowm", [4, RKT, 128, D // 4], F8,
                          kind="ExternalInput")
    # combine scalars, broadcast over B rows: [:, 0] = 64*bd[l0],
    # [:, 1] = 64*bd[l1].
    bdb = nc.dram_tensor("bdb", [B, 2], F32, kind="ExternalInput")
    # row-parallel partial-sum output (full width).
    out = nc.dram_tensor("out", [B, D], F32, kind="ExternalOutput")

    with tile.TileContext(nc) as tc:
        with (
            tc.tile_pool(name="wc", bufs=2 * NG) as wcpool,
            tc.tile_pool(name="wr", bufs=8 * RKT) as wrpool,
            tc.tile_pool(name="xp", bufs=1) as xpool,
            tc.tile_pool(name="sp", bufs=4) as spool,
            tc.tile_pool(name="const", bufs=1) as cpool,
            tc.tile_pool(name="ps", bufs=1, space="PSUM") as psum,
        ):
            bd_sb = cpool.tile([128, 2], F32, tag="bd")
            nc.scalar.dma_start(bd_sb[0:B, :], bdb[:, :])
            nc.scalar.dma_start(bd_sb[64:64 + B, :], bdb[:, :])
            ident = cpool.tile([128, B], F32, tag="ident")
            make_identity(nc, ident[0:B, :])
            make_identity(nc, ident[64:64 + B, :])

            # ---- PE warm-up: dummy matmuls while the first weight DMAs
            # are in flight, so HAM reaches K=8/8 before the real stream.
            warm = cpool.tile([128, B], BF16, tag="warm")
            nc.vector.memset(warm[:, :], 0.0)
            wps = psum.tile([B, B], F32, tag="wps")
            for _ in range(NWARM):
                nc.tensor.matmul(wps[:, :], warm[:, :], warm[:, :],
                                 start=True, stop=True)

            # xt[p, k*16+b] = x^T[p*KT + k, b]
            xt = xpool.tile([128, KT * B], BF16, tag="xt")
            nc.scalar.dma_start(
                xt[:, :].rearrange("p (k b) -> p k b", k=KT),
                xT0[:, :].rearrange("(p k) b -> p k b", p=128),
            )

            # weight DMAs: all on one hardware queue (a single queue is
            # fanned across all 16 DMA engines and sustains ~420 GB/s;
            # splitting across queues measured slower), in consumption
            # order: col groups, then row chunks quarter-major.
            wbt, wmt = [], []
            for g in range(NG):
                wb = wcpool.tile([128, GK * C], F8, tag="wcb")
                nc.sync.dma_start(wb[:, :], colb[g])
                wm = wcpool.tile([128, GK * C], F8, tag="wcm")
                nc.sync.dma_start(wm[:, :], colm[g])
                wbt.append(wb)
                wmt.append(wm)
            rbt, rmt = [], []
            for h in range(4):
                for kc in range(RKT):
                    rb = wrpool.tile([128, D // 4], F8, tag="wrb")
                    nc.sync.dma_start(rb[:, :], rowb[h, kc])
                    rm = wrpool.tile([128, D // 4], F8, tag="wrm")
                    nc.sync.dma_start(rm[:, :], rowm[h, kc])
                    rbt.append(rb)
                    rmt.append(rm)

            # ---- column-parallel layer: y0[16, 512] = x @ Wcol ----
            # 2-strip PE col tiling: strip s covers cols [s*256, s*256+256)
            # in PE col group 64*s, PSUM partitions [64*s, 64*s+16).
            # acc bank layout: b at cols [0,256), m at cols [256,512).
            cacc = psum.tile([128, 1024], F32, tag="cacc")  # b: bank0, m: bank1
            HC = 256
            for g in range(NG):
                for j in range(GK):
                    k = g * GK + j
                    lhsT = xt[:, k * B:(k + 1) * B]
                    for s in range(2):
                        p0 = 64 * s
                        nc.tensor.matmul(
                            cacc[p0:p0 + B, 0:HC], lhsT,
                            wbt[g][:, j * C + s * HC:j * C + (s + 1) * HC],
                            start=(k == 0), stop=(k == KT - 1),
                            tile_position=(0, p0), skip_group_check=True,
                        )
                        nc.tensor.matmul(
                            cacc[p0:p0 + B, 512:512 + HC], lhsT,
                            wmt[g][:, j * C + s * HC:j * C + (s + 1) * HC],
                            start=(k == 0), stop=(k == KT - 1),
                            tile_position=(0, p0), skip_group_check=True,
                        )

            # y0 = acc_b + (64*bd0) * acc_m, per strip (partitions 0/64)
            y0 = spool.tile([128, HC], F32, tag="y0")
            for s in range(2):
                p0 = 64 * s
                yb = spool.tile([128, HC], F32, tag="yb")
                nc.scalar.copy(yb[p0:p0 + B, :], cacc[p0:p0 + B, 0:HC])
                nc.vector.scalar_tensor_tensor(
                    out=y0[p0:p0 + B, :], in0=cacc[p0:p0 + B, 512:512 + HC],
                    scalar=bd_sb[p0:p0 + B, 0:1], in1=yb[p0:p0 + B, :],
                    op0=ALU.mult, op1=ALU.add,
                )

            # ---- local transpose: y0 -> xt1 [128, 4*16] ----
            # strip s holds y0 cols [s*256,(s+1)*256) on partitions 64s..
            tp = psum.tile([128, RKT * B], F32, tag="tp")
            for cc in range(RKT):
                s, half = divmod(cc, 2)
                p0 = 64 * s
                nc.tensor.transpose(
                    tp[:, cc * B:(cc + 1) * B],
                    y0[p0:p0 + B, half * 128:(half + 1) * 128],
                    ident[p0:p0 + B, :],
                )
            xt1 = spool.tile([128, RKT * B], BF16, tag="xt1")
            nc.vector.tensor_copy(xt1[:, :], tp[:, :])

            # ---- row-parallel layer: partial[16, 4096] = y0 @ Wrow ----
            # quarter h covers cols [h*1024, h*1024+1024): strip s does the
            # 512-chunk h*2+s in PE col group 64*s.  PSUM quarters
            # ping-pong (tags rq0/rq1) so quarter h+1 never waits on the
            # combine of quarter h.
            qacc = [
                psum.tile([128, C], F32, tag=f"rq{i}", name=f"rq{i}")
                for i in range(2)
            ]
            macc = [
                psum.tile([128, C], F32, tag=f"mq{i}", name=f"mq{i}")
                for i in range(2)
            ]
            for h in range(4):
                ab, am = qacc[h % 2], macc[h % 2]
                for kc in range(RKT):
                    lhsT = xt1[:, kc * B:(kc + 1) * B]
                    for s in range(2):
                        p0 = 64 * s
                        n0 = s * C
                        t = h * RKT + kc
                        nc.tensor.matmul(
                            ab[p0:p0 + B, :], lhsT, rbt[t][:, n0:n0 + C],
                            start=(kc == 0), stop=(kc == RKT - 1),
                            tile_position=(0, p0), skip_group_check=True,
                        )
                        nc.tensor.matmul(
                            am[p0:p0 + B, :], lhsT, rmt[t][:, n0:n0 + C],
                            start=(kc == 0), stop=(kc == RKT - 1),
                            tile_position=(0, p0), skip_group_check=True,
                        )
                for s in range(2):
                    p0 = 64 * s
                    no = (h * 2 + s) * C
                    yb2 = spool.tile([128, C], F32, tag="yb2")
                    nc.scalar.copy(yb2[p0:p0 + B, :], ab[p0:p0 + B, :])
                    y1 = spool.tile([128, C], F32, tag="y1")
                    nc.vector.scalar_tensor_tensor(
                        out=y1[p0:p0 + B, :], in0=am[p0:p0 + B, :],
                        scalar=bd_sb[p0:p0 + B, 1:2], in1=yb2[p0:p0 + B, :],
                        op0=ALU.mult, op1=ALU.add,
                    )
                    nc.scalar.dma_start(out[:, no:no + C], y1[p0:p0 + B, :])

    nc.compile()
    return nc


def _get_nc():
    if "nc" not in _cache:
        _cache["nc"] = build()
    return _cache["nc"]


def _fp8(a):
    import ml_dtypes

    return np.clip(a, -240.0, 240.0).astype(ml_dtypes.float8_e4m3)


def _shard_row(w):
    """[C, D] row shard -> [4, RKT, 128, D//4] with
    out[h, kc, p, m] = w[kc*128 + p, h*1024 + m]."""
    w = w.reshape(RKT, 128, 4, D // 4)
    return np.ascontiguousarray(w.transpose(2, 0, 1, 3))


def _shard_col(w):
    """[D, C] col shard -> [NG, 128, GK*C] with
    out[g, p, j*C + c] = w[p*KT + g*GK + j, c]."""
    w = w.reshape(128, NG, GK, C)
    return np.ascontiguousarray(w.transpose(1, 0, 2, 3).reshape(NG, 128, GK * C))


def _make_in_maps(xT, w0, w1, bd0, bd1):
    """One exec = col layer (w0) + row layer (w1). xT: [D, B]."""
    import ml_dtypes
    bdb = np.broadcast_to(
        np.array([bd0 * SCALE, bd1 * SCALE], np.float32)[None, :], (B, 2)
    ).copy()
    in_maps = []
    for c in range(NCORES):
        sl = slice(c * C, (c + 1) * C)
        in_maps.append({
            "xT0": np.ascontiguousarray(xT).astype(ml_dtypes.bfloat16),
            "colb": _shard_col(_fp8(w0[0][:, sl] * SCALE)),
            "colm": _shard_col(_fp8(w0[1][:, sl])),
            "rowb": _shard_row(_fp8(w1[0][sl, :] * SCALE)),
            "rowm": _shard_row(_fp8(w1[1][sl, :])),
            "bdb": bdb,
        })
    return in_maps


def _run(x, base, mask, bitdelta, trace=False):
    nc = _get_nc()
    base = np.asarray(base, np.float32)
    mask = np.asarray(mask, np.float32)
    bd = np.asarray(bitdelta, np.float32)

    results = []
    xT = np.ascontiguousarray(np.asarray(x, np.float32).T)
    for e in range(2):
        l0, l1 = 2 * e, 2 * e + 1
        in_maps = _make_in_maps(
            xT, (base[l0], mask[l0]), (base[l1], mask[l1]), bd[l0], bd[l1]
        )
        res = run_bass_kernel_spmd(
            nc, in_maps, core_ids=list(range(NCORES)), trace=trace
        )
        # unshard the row-parallel output: sum the 8 partial shards
        y = np.sum([res.results[c]["out"] for c in range(NCORES)], axis=0)
        results.append(res)
        xT = np.ascontiguousarray(y.T)
    return y / SCALE ** 4, results


def kernel(x, base, mask, bitdelta):
    y, _ = _run(x, base, mask, bitdelta)
    return y.astype(np.float32)


# revision 15
# speedup vs baseline: 2.3212x; 1.0538x over previous
"""Trainium2 Bass kernel for a 4-layer compressed model:

    for l in range(4):  x = x @ (base[l] + bitdelta[l] * mask[l])

x: [16, 4096] f32, base/mask: [4, 4096, 4096] f32, bitdelta: [4] f32.

Strategy (8 cores):
  * Weights stream as fp8e4: mask is a +/-1 sign tensor (EXACT in fp8);
    base is pre-scaled x64 on the host before fp8 quantization (keeps
    values out of the subnormal floor).  The x64 is folded into the
    per-layer combine scalar (64*bd) so every layer output is scaled by
    64; the host divides the final gather by 64^4.  This quarters HBM
    traffic vs f32 - the problem is memory-bound.
  * By linearity, x @ (base + bd*mask) = x @ base + bd * (x @ mask):
    base and mask stream straight into the PE as fp8 moving operands
    against a stationary x^T tile (bf16), accumulating into separate
    PSUM banks; one DVE scalar_tensor_tensor op combines them.
  * The network is linear end to end, so layers are paired
    column-parallel -> row-parallel: core c owns columns [c*512,(c+1)*512)
    of layer 2e and rows [c*512,(c+1)*512) of layer 2e+1.  The col->row
    hand-off is core-LOCAL (a 4x PE transpose, no DRAM round trip) and
    the row layer emits a [16, 4096] partial-sum shard.  There are NO
    on-device collectives (ncfw warmup costs ~55us/exec and AllGathers
    ~16us each - they would dominate at fp8 speed).  kernel() runs the
    same compiled program twice (layers 0/1 then 2/3); unsharding the
    row-parallel output (summing the 8 partials) happens on the host.
  * Perf details: weight DMAs are spread over four hardware queues
    (sync/vector/gpsimd/scalar) so the descriptor ramp is 4x faster; a
    block of dummy N=16 matmuls during the initial DMA wait keeps the
    PE HAM clock-gate at K=8/8 (2.4 GHz) before the real stream; every
    matmul is issued as a 2-strip pair in PE column groups 0/64
    (tile_position), so two moving streams run concurrently and the PE
    stays ahead of DMA; row-layer PSUM quarters ping-pong to avoid
    write-after-read stalls.
"""

import numpy as np

import concourse.bass as bass
import concourse.mybir as mybir
import concourse.tile as tile
from concourse import bacc
from concourse.bass_utils import run_bass_kernel_spmd
from concourse.masks import make_identity

D = 4096
B = 16
NCORES = 8
C = D // NCORES          # 512 cols (rows) per core per layer
KT = D // 128            # 32 contraction tiles in the col layer
GK = 4                   # col-layer k-tiles per weight DMA (256 KiB)
NG = KT // GK            # 8 weight DMAs per tensor for the col layer
RKT = C // 128           # 4 contraction tiles in the row layer
SCALE = 64.0             # host pre-scale on base before fp8 quantization
NWARM = 55               # PE warm-up matmuls (~4us to open the HAM gate)

F32 = mybir.dt.float32
F8 = mybir.dt.float8e4
BF16 = mybir.dt.bfloat16
ALU = mybir.AluOpType

_cache = {}


def build():
    nc = bacc.Bacc(
        "TRN2",
        target_bir_lowering=False,
        debug=False,
        num_devices=NCORES,
    )

    # x^T [4096, 16]; row d = p*KT + k maps to SBUF partition p, matmul
    # index k, so the load is partition-contiguous.
    xT0 = nc.dram_tensor("xT0", [D, B], BF16, kind="ExternalInput")
    # col-layer weight shards, host-permuted: [g, p, j*C+c] =
    # W[p*KT + g*GK + j, c0+c]; each [128, GK*C] block is one contiguous
    # 256 KiB DMA.
    colb = nc.dram_tensor("colb", [NG, 128, GK * C], F8, kind="ExternalInput")
    colm = nc.dram_tensor("colm", [NG, 128, GK * C], F8, kind="ExternalInput")
    # row-layer weight shards, quarter-major so the first output quarter
    # can start before the rest arrive, two kc-tiles packed per chunk so
    # each DMA moves 256 KiB with 2 KiB per-partition lines:
    # [h, j, p, jj*1024 + m] = W[c0 + (2j+jj)*128 + p, h*1024 + m].
    rowb = nc.dram_tensor("rowb", [4, RKT // 2, 128, D // 2], F8,
                          kind="ExternalInput")
    rowm = nc.dram_tensor("rowm", [4, RKT // 2, 128, D // 2], F8,
                          kind="ExternalInput")
    # combine scalars, broadcast over B rows: [:, 0] = 64*bd[l0],
    # [:, 1] = 64*bd[l1].
    bdb = nc.dram_tensor("bdb", [B, 2], F32, kind="ExternalInput")
    # row-parallel partial-sum output (full width).
    out = nc.dram_tensor("out", [B, D], F32, kind="ExternalOutput")

    with tile.TileContext(nc) as tc:
        with (
            tc.tile_pool(name="wc", bufs=2 * NG) as wcpool,
            tc.tile_pool(name="wr", bufs=4 * RKT) as wrpool,
            tc.tile_pool(name="xp", bufs=1) as xpool,
            tc.tile_pool(name="sp", bufs=4) as spool,
            tc.tile_pool(name="const", bufs=1) as cpool,
            tc.tile_pool(name="ps", bufs=1, space="PSUM") as psum,
        ):
            bd_sb = cpool.tile([128, 2], F32, tag="bd")
            nc.scalar.dma_start(bd_sb[0:B, :], bdb[:, :])
            nc.scalar.dma_start(bd_sb[64:64 + B, :], bdb[:, :])
            ident = cpool.tile([128, B], F32, tag="ident")
            make_identity(nc, ident[0:B, :])
            make_identity(nc, ident[64:64 + B, :])

            # ---- PE warm-up: dummy matmuls while the first weight DMAs
            # are in flight, so HAM reaches K=8/8 before the real stream.
            warm = cpool.tile([128, B], BF16, tag="warm")
            nc.vector.memset(warm[:, :], 0.0)
            wps = psum.tile([B, B], F32, tag="wps")
            for _ in range(NWARM):
                nc.tensor.matmul(wps[:, :], warm[:, :], warm[:, :],
                                 start=True, stop=True)

            # xt[p, k*16+b] = x^T[p*KT + k, b]
            xt = xpool.tile([128, KT * B], BF16, tag="xt")
            nc.scalar.dma_start(
                xt[:, :].rearrange("p (k b) -> p k b", k=KT),
                xT0[:, :].rearrange("(p k) b -> p k b", p=128),
            )

            # weight DMAs: all on one hardware queue (a single queue is
            # fanned across all 16 DMA engines and sustains ~420 GB/s;
            # splitting across queues measured slower), in consumption
            # order: col groups, then row chunks quarter-major.
            wbt, wmt = [], []
            for g in range(NG):
                wb = wcpool.tile([128, GK * C], F8, tag="wcb")
                nc.sync.dma_start(wb[:, :], colb[g])
                wm = wcpool.tile([128, GK * C], F8, tag="wcm")
                nc.sync.dma_start(wm[:, :], colm[g])
                wbt.append(wb)
                wmt.append(wm)
            rbt, rmt = [], []
            for h in range(4):
                for j in range(RKT // 2):
                    rb = wrpool.tile([128, D // 2], F8, tag="wrb")
                    nc.sync.dma_start(rb[:, :], rowb[h, j])
                    rm = wrpool.tile([128, D // 2], F8, tag="wrm")
                    nc.sync.dma_start(rm[:, :], rowm[h, j])
                    rbt.append(rb)
                    rmt.append(rm)

            # ---- column-parallel layer: y0[16, 512] = x @ Wcol ----
            # 2-strip PE col tiling: strip s covers cols [s*256, s*256+256)
            # in PE col group 64*s, PSUM partitions [64*s, 64*s+16).
            # acc bank layout: b at cols [0,256), m at cols [256,512).
            cacc = psum.tile([128, 1024], F32, tag="cacc")  # b: bank0, m: bank1
            HC = 256
            for g in range(NG):
                for j in range(GK):
                    k = g * GK + j
                    lhsT = xt[:, k * B:(k + 1) * B]
                    for s in range(2):
                        p0 = 64 * s
                        nc.tensor.matmul(
                            cacc[p0:p0 + B, 0:HC], lhsT,
                            wbt[g][:, j * C + s * HC:j * C + (s + 1) * HC],
                            start=(k == 0), stop=(k == KT - 1),
                            tile_position=(0, p0), skip_group_check=True,
                        )
                        nc.tensor.matmul(
                            cacc[p0:p0 + B, 512:512 + HC], lhsT,
                            wmt[g][:, j * C + s * HC:j * C + (s + 1) * HC],
                            start=(k == 0), stop=(k == KT - 1),
                            tile_position=(0, p0), skip_group_check=True,
                        )

            # y0 = acc_b + (64*bd0) * acc_m, per strip (partitions 0/64)
            y0 = spool.tile([128, HC], F32, tag="y0")
            for s in range(2):
                p0 = 64 * s
                yb = spool.tile([128, HC], F32, tag="yb")
                nc.scalar.copy(yb[p0:p0 + B, :], cacc[p0:p0 + B, 0:HC])
                nc.vector.scalar_tensor_tensor(
                    out=y0[p0:p0 + B, :], in0=cacc[p0:p0 + B, 512:512 + HC],
                    scalar=bd_sb[p0:p0 + B, 0:1], in1=yb[p0:p0 + B, :],
                    op0=ALU.mult, op1=ALU.add,
                )

            # ---- local transpose: y0 -> xt1 [128, 4*16] ----
            # strip s holds y0 cols [s*256,(s+1)*256) on partitions 64s..
            tp = psum.tile([128, RKT * B], F32, tag="tp")
            for cc in range(RKT):
                s, half = divmod(cc, 2)
                p0 = 64 * s
                nc.tensor.transpose(
                    tp[:, cc * B:(cc + 1) * B],
                    y0[p0:p0 + B, half * 128:(half + 1) * 128],
                    ident[p0:p0 + B, :],
                )
            xt1 = spool.tile([128, RKT * B], BF16, tag="xt1")
            nc.vector.tensor_copy(xt1[:, :], tp[:, :])

            # ---- row-parallel layer: partial[16, 4096] = y0 @ Wrow ----
            # quarter h covers cols [h*1024, h*1024+1024): strip s does the
            # 512-chunk h*2+s in PE col group 64*s.  PSUM quarters
            # ping-pong (tags rq0/rq1) so quarter h+1 never waits on the
            # combine of quarter h.
            qacc = [
                psum.tile([128, C], F32, tag=f"rq{i}", name=f"rq{i}")
                for i in range(2)
            ]
            macc = [
                psum.tile([128, C], F32, tag=f"mq{i}", name=f"mq{i}")
                for i in range(2)
            ]
            for h in range(4):
                ab, am = qacc[h % 2], macc[h % 2]
                for kc in range(RKT):
                    lhsT = xt1[:, kc * B:(kc + 1) * B]
                    for s in range(2):
                        p0 = 64 * s
                        t = h * (RKT // 2) + kc // 2
                        n0 = (kc % 2) * (D // 4) + s * C
                        nc.tensor.matmul(
                            ab[p0:p0 + B, :], lhsT, rbt[t][:, n0:n0 + C],
                            start=(kc == 0), stop=(kc == RKT - 1),
                            tile_position=(0, p0), skip_group_check=True,
                        )
                        nc.tensor.matmul(
                            am[p0:p0 + B, :], lhsT, rmt[t][:, n0:n0 + C],
                            start=(kc == 0), stop=(kc == RKT - 1),
                            tile_position=(0, p0), skip_group_check=True,
                        )
                for s in range(2):
                    p0 = 64 * s
                    no = (h * 2 + s) * C
                    yb2 = spool.tile([128, C], F32, tag="yb2")
                    nc.scalar.copy(yb2[p0:p0 + B, :], ab[p0:p0 + B, :])
                    y1 = spool.tile([128, C], F32, tag="y1")
                    nc.vector.scalar_tensor_tensor(
                        out=y1[p0:p0 + B, :], in0=am[p0:p0 + B, :],
                        scalar=bd_sb[p0:p0 + B, 1:2], in1=yb2[p0:p0 + B, :],
                        op0=ALU.mult, op1=ALU.add,
                    )
                    nc.scalar.dma_start(out[:, no:no + C], y1[p0:p0 + B, :])

    nc.compile()
    return nc


def _get_nc():
    if "nc" not in _cache:
        _cache["nc"] = build()
    return _cache["nc"]


def _fp8(a):
    import ml_dtypes

    return np.clip(a, -240.0, 240.0).astype(ml_dtypes.float8_e4m3)


def _shard_row(w):
    """[C, D] row shard -> [4, RKT//2, 128, D//2] with
    out[h, j, p, jj*1024 + m] = w[(2j+jj)*128 + p, h*1024 + m]."""
    w = w.reshape(RKT // 2, 2, 128, 4, D // 4)      # [j, jj, p, h, m]
    w = w.transpose(3, 0, 2, 1, 4)                  # [h, j, p, jj, m]
    return np.ascontiguousarray(w.reshape(4, RKT // 2, 128, D // 2))


def _shard_col(w):
    """[D, C] col shard -> [NG, 128, GK*C] with
    out[g, p, j*C + c] = w[p*KT + g*GK + j, c]."""
    w = w.reshape(128, NG, GK, C)
    return np.ascontiguousarray(w.transpose(1, 0, 2, 3).reshape(NG, 128, GK * C))


def _make_in_maps(xT, w0, w1, bd0, bd1):
    """One exec = col layer (w0) + row layer (w1). xT: [D, B]."""
    import ml_dtypes
    bdb = np.broadcast_to(
        np.array([bd0 * SCALE, bd1 * SCALE], np.float32)[None, :], (B, 2)
    ).copy()
    in_maps = []
    for c in range(NCORES):
        sl = slice(c * C, (c + 1) * C)
        in_maps.append({
            "xT0": np.ascontiguousarray(xT).astype(ml_dtypes.bfloat16),
            "colb": _shard_col(_fp8(w0[0][:, sl] * SCALE)),
            "colm": _shard_col(_fp8(w0[1][:, sl])),
            "rowb": _shard_row(_fp8(w1[0][sl, :] * SCALE)),
            "rowm": _shard_row(_fp8(w1[1][sl, :])),
            "bdb": bdb,
        })
    return in_maps


def _run(x, base, mask, bitdelta, trace=False):
    nc = _get_nc()
    base = np.asarray(base, np.float32)
    mask = np.asarray(mask, np.float32)
    bd = np.asarray(bitdelta, np.float32)

    results = []
    xT = np.ascontiguousarray(np.asarray(x, np.float32).T)
    for e in range(2):
        l0, l1 = 2 * e, 2 * e + 1
        in_maps = _make_in_maps(
            xT, (base[l0], mask[l0]), (base[l1], mask[l1]), bd[l0], bd[l1]
        )
        res = run_bass_kernel_spmd(
            nc, in_maps, core_ids=list(range(NCORES)), trace=trace
        )
        # unshard the row-parallel output: sum the 8 partial shards
        y = np.sum([res.results[c]["out"] for c in range(NCORES)], axis=0)
        results.append(res)
        xT = np.ascontiguousarray(y.T)
    return y / SCALE ** 4, results


def kernel(x, base, mask, bitdelta):
    y, _ = _run(x, base, mask, bitdelta)
    return y.astype(np.float32)
